# revision 12
# baseline (speedup 1.0000x reference)
"""GRU cell (B=4096, H=2048) on 8 TRN2 NeuronCores.

Sharding: data-parallel over the batch dim — each core computes 512 rows.
Weights are replicated; no collectives.

Per-core compute runs in "transposed" space (hidden on partitions, batch on
the free dim): for hidden block nb (128 units) the three gate pre-activations
are built by PSUM accumulation

    psum = sum_k W[k*128:(k+1)*128, nb*128:(nb+1)*128]^T @ actT[k]

with float32r (FP22 reduced-precision fp32) matmuls at free-dim 512, which run
at full PE rate. r/z gates accumulate the ih and hh contributions into a
single PSUM bank; the n gate keeps gi2/gh2 separate (needed for gi2 + r*gh2).
Biases become per-partition scalars in this layout, so ScalarE fuses them into
the sigmoid/tanh activation. The host pre-transposes the activation shards and
packs the weights so every weight DMA is one contiguous 1 MiB slab.
"""

from contextlib import ExitStack

import ml_dtypes
import numpy as np

import concourse.bass as bass
import concourse.tile as tile
from concourse import bacc, mybir
from concourse.bass_utils import run_bass_kernel_spmd

H = 2048
B = 4096
N_CORES = 8
BL = B // N_CORES  # 512 batch rows per core
P = 128
NKB = H // P  # 16 contraction chunks
NNB = H // P  # 16 hidden (output) blocks
F32 = mybir.dt.float32
F32R = mybir.dt.float32r
BF16 = mybir.dt.bfloat16

# Weight matrix order in the packed tensor: (gate, ih/hh)
# 0: W_ih[0] (r)   1: W_hh[0] (r)
# 2: W_ih[1] (z)   3: W_hh[1] (z)
# 4: W_ih[2] (n)   5: W_hh[2] (n)


def _build_program() -> bacc.Bacc:
    nc = bacc.Bacc(
        "TRN2", target_bir_lowering=False, debug=False, num_devices=N_CORES
    )

    # float32r (reduced-precision fp32) end-to-end on the matmul operand
    # path: the BIR verifier requires matmul inputs to be produced as f32r.
    # Same bits as f32; numpy binding is float32.
    xt = nc.dram_tensor("xt", [P, NKB * BL], F32R, kind="ExternalInput").ap()
    hxt = nc.dram_tensor("hxt", [P, NKB * BL], F32R, kind="ExternalInput").ap()
    w = nc.dram_tensor("w", [6, NNB, P, H], F32R, kind="ExternalInput").ap()
    b = nc.dram_tensor("b", [P, 4 * NNB], F32, kind="ExternalInput").ap()
    out = nc.dram_tensor("out", [H, BL], F32, kind="ExternalOutput").ap()

    with tile.TileContext(nc) as tc, ExitStack() as ctx:
        const = ctx.enter_context(tc.tile_pool(name="const", bufs=1))
        acts = ctx.enter_context(tc.tile_pool(name="acts", bufs=1))
        wpool = ctx.enter_context(tc.tile_pool(name="wpool", bufs=10))
        gates = ctx.enter_context(tc.tile_pool(name="gates", bufs=2))
        opool = ctx.enter_context(tc.tile_pool(name="opool", bufs=3))
        ps_r = ctx.enter_context(tc.tile_pool(name="ps_r", bufs=2, space="PSUM"))
        ps_z = ctx.enter_context(tc.tile_pool(name="ps_z", bufs=2, space="PSUM"))
        ps_gi = ctx.enter_context(tc.tile_pool(name="ps_gi", bufs=2, space="PSUM"))
        ps_gh = ctx.enter_context(tc.tile_pool(name="ps_gh", bufs=2, space="PSUM"))

        # Startup: the sync ring delivers the critical path in need order
        # (xt chunk 0 -> w0 -> rest of xt -> w2 -> w4 -> nb>=1 slabs); the
        # scalar ring concurrently brings biases, hxt and nb0's hh slabs.
        btile = const.tile([P, 4 * NNB], F32)
        nc.scalar.dma_start(btile[:], b[:])
        xt_sb = acts.tile([P, NKB * BL], F32R)
        hxt_sb = acts.tile([P, NKB * BL], F32R)
        nb0_slabs = [None] * 6
        CH = 4 * BL  # 1 MiB activation chunks (4 k-blocks each)
        nc.sync.dma_start(xt_sb[:, 0:CH], xt[:, 0:CH])
        s = wpool.tile([P, H], F32R, tag="wslab", name="w0_0")
        nc.sync.dma_start(s[:], w[0, 0])
        nb0_slabs[0] = s
        for c in range(1, 4):
            nc.sync.dma_start(
                xt_sb[:, c * CH : (c + 1) * CH], xt[:, c * CH : (c + 1) * CH]
            )
        for m in (2, 4):
            s = wpool.tile([P, H], F32R, tag="wslab", name=f"w{m}_0")
            nc.sync.dma_start(s[:], w[m, 0])
            nb0_slabs[m] = s
        nc.scalar.dma_start(hxt_sb[:], hxt[:])
        for m in (1, 3, 5):
            s = wpool.tile([P, H], F32R, tag="wslab", name=f"w{m}_0")
            nc.scalar.dma_start(s[:], w[m, 0])
            nb0_slabs[m] = s

        # PE warm-up: throwaway matmuls on a memset tile release the HAM
        # clock gate and keep the PE busy while the startup DMAs land, so
        # the first real matmuls run at 2.4 GHz.
        warm = const.tile([P, BL], BF16)
        nc.gpsimd.memset(warm[:], 0.0)
        p_warm = ps_gh.tile([P, BL], F32, tag="p_gh", name="p_warm")

        def warm_mms(n):
            for _ in range(n):
                nc.tensor.matmul(
                    p_warm[:], lhsT=warm[:, :P], rhs=warm[:],
                    start=True, stop=True,
                )

        warm_mms(20)

        def mm_half(psum, slab, act_sb, start, stop):
            """One 16-matmul K sweep accumulated into psum."""
            for k in range(NKB):
                nc.tensor.matmul(
                    psum[:],
                    lhsT=slab[:, k * P : (k + 1) * P],
                    rhs=act_sb[:, k * BL : (k + 1) * BL],
                    start=(start and k == 0),
                    stop=(stop and k == NKB - 1),
                )

        for nb in range(NNB):
            sl = [None] * 6
            order = (0, 2, 4, 1, 3, 5) if nb == 0 else (4, 5, 0, 1, 2, 3)
            for m in order:
                if nb == 0:
                    sl[m] = nb0_slabs[m]
                    continue
                s = wpool.tile([P, H], F32R, tag="wslab", name=f"w{m}_{nb}")
                nc.sync.dma_start(s[:], w[m, nb])
                sl[m] = s

            p_r = ps_r.tile([P, BL], F32)
            p_z = ps_z.tile([P, BL], F32)
            p_gi = ps_gi.tile([P, BL], F32)
            p_gh = ps_gh.tile([P, BL], F32)
            if nb == 0:
                # xt-only halves first so the PE can start before hxt lands
                mm_half(p_r, sl[0], xt_sb, start=True, stop=False)
                mm_half(p_z, sl[2], xt_sb, start=True, stop=False)
                mm_half(p_gi, sl[4], xt_sb, start=True, stop=True)
                warm_mms(6)
                mm_half(p_r, sl[1], hxt_sb, start=False, stop=True)
                mm_half(p_z, sl[3], hxt_sb, start=False, stop=True)
                mm_half(p_gh, sl[5], hxt_sb, start=True, stop=True)
            else:
                # n-gate first: its tanh chain overlaps the r/z matmuls,
                # leaving only sigmoid -> mul -> add after the last matmul.
                mm_half(p_gi, sl[4], xt_sb, start=True, stop=True)
                mm_half(p_gh, sl[5], hxt_sb, start=True, stop=True)
                mm_half(p_r, sl[0], xt_sb, start=True, stop=False)
                mm_half(p_r, sl[1], hxt_sb, start=False, stop=True)
                mm_half(p_z, sl[2], xt_sb, start=True, stop=False)
                mm_half(p_z, sl[3], hxt_sb, start=False, stop=True)

            def bias_ap(g):
                return btile[:, g * NNB + nb : g * NNB + nb + 1]

            # r = sigmoid(gi0 + gh0 + b_ih0 + b_hh0)
            r_sb = gates.tile([P, BL], F32, tag="r")
            nc.scalar.activation(
                r_sb[:], p_r[:], mybir.ActivationFunctionType.Sigmoid,
                bias=bias_ap(0),
            )
            # z = sigmoid(gi1 + gh1 + b_ih1 + b_hh1)
            z_sb = gates.tile([P, BL], F32, tag="z")
            nc.scalar.activation(
                z_sb[:], p_z[:], mybir.ActivationFunctionType.Sigmoid,
                bias=bias_ap(1),
            )
            # t = (gh2 + b_hh2) * r
            t_sb = gates.tile([P, BL], F32, tag="t")
            nc.vector.scalar_tensor_tensor(
                t_sb[:], p_gh[:], bias_ap(3), r_sb[:],
                op0=mybir.AluOpType.add, op1=mybir.AluOpType.mult,
            )
            # n = tanh(gi2 + b_ih2 + t)
            x_sb = gates.tile([P, BL], F32, tag="x")
            nc.vector.tensor_add(x_sb[:], t_sb[:], p_gi[:])
            n_sb = gates.tile([P, BL], F32, tag="n")
            nc.scalar.activation(
                n_sb[:], x_sb[:], mybir.ActivationFunctionType.Tanh,
                bias=bias_ap(2),
            )
            # out = n + z * (hx - n)
            d_sb = gates.tile([P, BL], F32, tag="d")
            nc.vector.tensor_sub(
                d_sb[:], hxt_sb[:, nb * BL : (nb + 1) * BL].bitcast(F32), n_sb[:]
            )
            e_sb = gates.tile([P, BL], F32, tag="e")
            nc.vector.tensor_mul(e_sb[:], z_sb[:], d_sb[:])
            o_sb = opool.tile([P, BL], F32, tag="o")
            nc.vector.tensor_add(o_sb[:], n_sb[:], e_sb[:])
            nc.gpsimd.dma_start(out[nb * P : (nb + 1) * P, :], o_sb[:])

    nc.compile()
    return nc


def _pack_inputs(input, hx, weight_ih, weight_hh, bias_ih, bias_hh):
    """Host-side shard + layout packing. Returns per-core input maps."""
    input = np.ascontiguousarray(np.asarray(input, dtype=np.float32))
    hx = np.ascontiguousarray(np.asarray(hx, dtype=np.float32))
    weight_ih = np.asarray(weight_ih, dtype=np.float32)
    weight_hh = np.asarray(weight_hh, dtype=np.float32)
    bias_ih = np.asarray(bias_ih, dtype=np.float32)
    bias_hh = np.asarray(bias_hh, dtype=np.float32)

    # wpack[m, nb, kp, k*128+n] = W_m[k*128+kp, nb*128+n]
    ws = [weight_ih[0], weight_hh[0], weight_ih[1], weight_hh[1],
          weight_ih[2], weight_hh[2]]
    wpack = np.ascontiguousarray(
        np.stack(
            [wm.reshape(NKB, P, NNB, P).transpose(2, 1, 0, 3) for wm in ws]
        ).reshape(6, NNB, P, H)
    )

    # bpack[p, g*16+nb] = bias_g[nb*128+p];  g order: r_sum, z_sum, ih2, hh2
    bias_all = np.stack(
        [bias_ih[0] + bias_hh[0], bias_ih[1] + bias_hh[1], bias_ih[2], bias_hh[2]]
    )  # [4, H]
    bpack = np.ascontiguousarray(
        bias_all.reshape(4, NNB, P).transpose(2, 0, 1).reshape(P, 4 * NNB)
    )

    def t_pack(a, dt=np.float32):
        # [BL, H] -> [P, NKB*BL] with [kp, k*BL+m] = a[m, k*128+kp]
        return np.ascontiguousarray(
            a.T.reshape(NKB, P, BL).transpose(1, 0, 2).reshape(P, NKB * BL)
            .astype(dt)
        )

    in_maps = []
    for c in range(N_CORES):
        sl = slice(c * BL, (c + 1) * BL)
        in_maps.append(
            {
                "xt": t_pack(input[sl]),
                "hxt": t_pack(hx[sl]),
                "w": wpack,
                "b": bpack,
            }
        )
    return in_maps


_PROGRAM_CACHE = []


def kernel(input, hx, weight_ih, weight_hh, bias_ih, bias_hh, _trace=False):
    if not _PROGRAM_CACHE:
        _PROGRAM_CACHE.append(_build_program())
    nc = _PROGRAM_CACHE[0]
    in_maps = _pack_inputs(input, hx, weight_ih, weight_hh, bias_ih, bias_hh)
    res = run_bass_kernel_spmd(nc, in_maps, list(range(N_CORES)), trace=_trace)
    out = np.empty((B, H), dtype=np.float32)
    for c in range(N_CORES):
        out[c * BL : (c + 1) * BL] = res.results[c]["out"].T
    if _trace:
        kernel.last_exec_time_ns = res.exec_time_ns
    return out


# revision 13
# speedup vs baseline: 1.0290x; 1.0290x over previous
"""GRU cell (B=4096, H=2048) on 8 TRN2 NeuronCores.

Sharding: data-parallel over the batch dim — each core computes 512 rows.
Weights are replicated; no collectives.

Per-core compute runs in "transposed" space (hidden on partitions, batch on
the free dim): for hidden block nb (128 units) the three gate pre-activations
are built by PSUM accumulation

    psum = sum_k W[k*128:(k+1)*128, nb*128:(nb+1)*128]^T @ actT[k]

with float32r (FP22 reduced-precision fp32) matmuls at free-dim 512, which run
at full PE rate. r/z gates accumulate the ih and hh contributions into a
single PSUM bank; the n gate keeps gi2/gh2 separate (needed for gi2 + r*gh2).
Biases become per-partition scalars in this layout, so ScalarE fuses them into
the sigmoid/tanh activation. The host pre-transposes the activation shards and
packs the weights so every weight DMA is one contiguous 1 MiB slab.
"""

from contextlib import ExitStack

import ml_dtypes
import numpy as np

import concourse.bass as bass
import concourse.tile as tile
from concourse import bacc, mybir
from concourse.bass_utils import run_bass_kernel_spmd

H = 2048
B = 4096
N_CORES = 8
BL = B // N_CORES  # 512 batch rows per core
P = 128
NKB = H // P  # 16 contraction chunks
NNB = H // P  # 16 hidden (output) blocks
F32 = mybir.dt.float32
F32R = mybir.dt.float32r
BF16 = mybir.dt.bfloat16

# Weight matrix order in the packed tensor: (gate, ih/hh)
# 0: W_ih[0] (r)   1: W_hh[0] (r)
# 2: W_ih[1] (z)   3: W_hh[1] (z)
# 4: W_ih[2] (n)   5: W_hh[2] (n)


def _build_program() -> bacc.Bacc:
    nc = bacc.Bacc(
        "TRN2", target_bir_lowering=False, debug=False, num_devices=N_CORES
    )

    # float32r (reduced-precision fp32) end-to-end on the matmul operand
    # path: the BIR verifier requires matmul inputs to be produced as f32r.
    # Same bits as f32; numpy binding is float32.
    xt = nc.dram_tensor("xt", [P, NKB * BL], F32R, kind="ExternalInput").ap()
    hxt = nc.dram_tensor("hxt", [P, NKB * BL], F32R, kind="ExternalInput").ap()
    w = nc.dram_tensor("w", [6, NNB, P, H], F32R, kind="ExternalInput").ap()
    b = nc.dram_tensor("b", [P, 4 * NNB], F32, kind="ExternalInput").ap()
    out = nc.dram_tensor("out", [H, BL], F32, kind="ExternalOutput").ap()

    with tile.TileContext(nc) as tc, ExitStack() as ctx:
        const = ctx.enter_context(tc.tile_pool(name="const", bufs=1))
        acts = ctx.enter_context(tc.tile_pool(name="acts", bufs=1))
        wpool = ctx.enter_context(tc.tile_pool(name="wpool", bufs=10))
        gates = ctx.enter_context(tc.tile_pool(name="gates", bufs=2))
        opool = ctx.enter_context(tc.tile_pool(name="opool", bufs=3))
        ps_r = ctx.enter_context(tc.tile_pool(name="ps_r", bufs=2, space="PSUM"))
        ps_z = ctx.enter_context(tc.tile_pool(name="ps_z", bufs=2, space="PSUM"))
        ps_gi = ctx.enter_context(tc.tile_pool(name="ps_gi", bufs=2, space="PSUM"))
        ps_gh = ctx.enter_context(tc.tile_pool(name="ps_gh", bufs=2, space="PSUM"))

        # Startup: one serial need-ordered stream on the sync ring — total
        # startup bytes are HBM-bound, so parallel rings only reshuffle the
        # stalls; serial delivery in consumption order minimizes them.
        btile = const.tile([P, 4 * NNB], F32)
        nc.scalar.dma_start(btile[:], b[:])
        xt_sb = acts.tile([P, NKB * BL], F32R)
        hxt_sb = acts.tile([P, NKB * BL], F32R)
        nb0_slabs = [None] * 6
        CH = 8 * BL  # 2 MiB activation chunks (8 k-blocks each)
        for c in range(2):
            nc.sync.dma_start(
                xt_sb[:, c * CH : (c + 1) * CH], xt[:, c * CH : (c + 1) * CH]
            )
        for m in (0, 2, 4):
            s = wpool.tile([P, H], F32R, tag="wslab", name=f"w{m}_0")
            nc.sync.dma_start(s[:], w[m, 0])
            nb0_slabs[m] = s
        for c in range(2):
            nc.sync.dma_start(
                hxt_sb[:, c * CH : (c + 1) * CH], hxt[:, c * CH : (c + 1) * CH]
            )
        for m in (1, 3, 5):
            s = wpool.tile([P, H], F32R, tag="wslab", name=f"w{m}_0")
            nc.sync.dma_start(s[:], w[m, 0])
            nb0_slabs[m] = s

        # PE warm-up: throwaway matmuls on a memset tile release the HAM
        # clock gate and keep the PE busy while the startup DMAs land, so
        # the first real matmuls run at 2.4 GHz.
        warm = const.tile([P, BL], BF16)
        nc.gpsimd.memset(warm[:], 0.0)
        p_warm = ps_gh.tile([P, BL], F32, tag="p_gh", name="p_warm")

        def warm_mms(n):
            for _ in range(n):
                nc.tensor.matmul(
                    p_warm[:], lhsT=warm[:, :P], rhs=warm[:],
                    start=True, stop=True,
                )

        warm_mms(45)

        def mm_half(psum, slab, act_sb, start, stop):
            """One 16-matmul K sweep accumulated into psum."""
            for k in range(NKB):
                nc.tensor.matmul(
                    psum[:],
                    lhsT=slab[:, k * P : (k + 1) * P],
                    rhs=act_sb[:, k * BL : (k + 1) * BL],
                    start=(start and k == 0),
                    stop=(stop and k == NKB - 1),
                )

        for nb in range(NNB):
            sl = [None] * 6
            order = (0, 2, 4, 1, 3, 5) if nb == 0 else (4, 5, 0, 1, 2, 3)
            for m in order:
                if nb == 0:
                    sl[m] = nb0_slabs[m]
                    continue
                s = wpool.tile([P, H], F32R, tag="wslab", name=f"w{m}_{nb}")
                nc.sync.dma_start(s[:], w[m, nb])
                sl[m] = s

            p_r = ps_r.tile([P, BL], F32)
            p_z = ps_z.tile([P, BL], F32)
            p_gi = ps_gi.tile([P, BL], F32)
            p_gh = ps_gh.tile([P, BL], F32)
            if nb == 0:
                # xt-only halves first so the PE can start before hxt lands
                mm_half(p_r, sl[0], xt_sb, start=True, stop=False)
                mm_half(p_z, sl[2], xt_sb, start=True, stop=False)
                mm_half(p_gi, sl[4], xt_sb, start=True, stop=True)
                warm_mms(6)
                mm_half(p_r, sl[1], hxt_sb, start=False, stop=True)
                mm_half(p_z, sl[3], hxt_sb, start=False, stop=True)
                mm_half(p_gh, sl[5], hxt_sb, start=True, stop=True)
            else:
                # n-gate first: its tanh chain overlaps the r/z matmuls,
                # leaving only sigmoid -> mul -> add after the last matmul.
                mm_half(p_gi, sl[4], xt_sb, start=True, stop=True)
                mm_half(p_gh, sl[5], hxt_sb, start=True, stop=True)
                mm_half(p_r, sl[0], xt_sb, start=True, stop=False)
                mm_half(p_r, sl[1], hxt_sb, start=False, stop=True)
                mm_half(p_z, sl[2], xt_sb, start=True, stop=False)
                mm_half(p_z, sl[3], hxt_sb, start=False, stop=True)

            def bias_ap(g):
                return btile[:, g * NNB + nb : g * NNB + nb + 1]

            # r = sigmoid(gi0 + gh0 + b_ih0 + b_hh0)
            r_sb = gates.tile([P, BL], F32, tag="r")
            nc.scalar.activation(
                r_sb[:], p_r[:], mybir.ActivationFunctionType.Sigmoid,
                bias=bias_ap(0),
            )
            # z = sigmoid(gi1 + gh1 + b_ih1 + b_hh1)
            z_sb = gates.tile([P, BL], F32, tag="z")
            nc.scalar.activation(
                z_sb[:], p_z[:], mybir.ActivationFunctionType.Sigmoid,
                bias=bias_ap(1),
            )
            # t = (gh2 + b_hh2) * r
            t_sb = gates.tile([P, BL], F32, tag="t")
            nc.vector.scalar_tensor_tensor(
                t_sb[:], p_gh[:], bias_ap(3), r_sb[:],
                op0=mybir.AluOpType.add, op1=mybir.AluOpType.mult,
            )
            # n = tanh(gi2 + b_ih2 + t)
            x_sb = gates.tile([P, BL], F32, tag="x")
            nc.vector.tensor_add(x_sb[:], t_sb[:], p_gi[:])
            n_sb = gates.tile([P, BL], F32, tag="n")
            nc.scalar.activation(
                n_sb[:], x_sb[:], mybir.ActivationFunctionType.Tanh,
                bias=bias_ap(2),
            )
            # out = n + z * (hx - n)
            d_sb = gates.tile([P, BL], F32, tag="d")
            nc.vector.tensor_sub(
                d_sb[:], hxt_sb[:, nb * BL : (nb + 1) * BL].bitcast(F32), n_sb[:]
            )
            e_sb = gates.tile([P, BL], F32, tag="e")
            nc.vector.tensor_mul(e_sb[:], z_sb[:], d_sb[:])
            o_sb = opool.tile([P, BL], F32, tag="o")
            nc.vector.tensor_add(o_sb[:], n_sb[:], e_sb[:])
            nc.gpsimd.dma_start(out[nb * P : (nb + 1) * P, :], o_sb[:])

    nc.compile()
    return nc


def _pack_inputs(input, hx, weight_ih, weight_hh, bias_ih, bias_hh):
    """Host-side shard + layout packing. Returns per-core input maps."""
    input = np.ascontiguousarray(np.asarray(input, dtype=np.float32))
    hx = np.ascontiguousarray(np.asarray(hx, dtype=np.float32))
    weight_ih = np.asarray(weight_ih, dtype=np.float32)
    weight_hh = np.asarray(weight_hh, dtype=np.float32)
    bias_ih = np.asarray(bias_ih, dtype=np.float32)
    bias_hh = np.asarray(bias_hh, dtype=np.float32)

    # wpack[m, nb, kp, k*128+n] = W_m[k*128+kp, nb*128+n]
    ws = [weight_ih[0], weight_hh[0], weight_ih[1], weight_hh[1],
          weight_ih[2], weight_hh[2]]
    wpack = np.ascontiguousarray(
        np.stack(
            [wm.reshape(NKB, P, NNB, P).transpose(2, 1, 0, 3) for wm in ws]
        ).reshape(6, NNB, P, H)
    )

    # bpack[p, g*16+nb] = bias_g[nb*128+p];  g order: r_sum, z_sum, ih2, hh2
    bias_all = np.stack(
        [bias_ih[0] + bias_hh[0], bias_ih[1] + bias_hh[1], bias_ih[2], bias_hh[2]]
    )  # [4, H]
    bpack = np.ascontiguousarray(
        bias_all.reshape(4, NNB, P).transpose(2, 0, 1).reshape(P, 4 * NNB)
    )

    def t_pack(a, dt=np.float32):
        # [BL, H] -> [P, NKB*BL] with [kp, k*BL+m] = a[m, k*128+kp]
        return np.ascontiguousarray(
            a.T.reshape(NKB, P, BL).transpose(1, 0, 2).reshape(P, NKB * BL)
            .astype(dt)
        )

    in_maps = []
    for c in range(N_CORES):
        sl = slice(c * BL, (c + 1) * BL)
        in_maps.append(
            {
                "xt": t_pack(input[sl]),
                "hxt": t_pack(hx[sl]),
                "w": wpack,
                "b": bpack,
            }
        )
    return in_maps


_PROGRAM_CACHE = []


def kernel(input, hx, weight_ih, weight_hh, bias_ih, bias_hh, _trace=False):
    if not _PROGRAM_CACHE:
        _PROGRAM_CACHE.append(_build_program())
    nc = _PROGRAM_CACHE[0]
    in_maps = _pack_inputs(input, hx, weight_ih, weight_hh, bias_ih, bias_hh)
    res = run_bass_kernel_spmd(nc, in_maps, list(range(N_CORES)), trace=_trace)
    out = np.empty((B, H), dtype=np.float32)
    for c in range(N_CORES):
        out[c * BL : (c + 1) * BL] = res.results[c]["out"].T
    if _trace:
        kernel.last_exec_time_ns = res.exec_time_ns
    return out
